# revision 14
# baseline (speedup 1.0000x reference)
"""Multi-head causal self-attention (B=2, S=2048, D=1024, H=16) on 8 TRN2 cores.

Sharding: core c handles batch b = c//4 and head group g = c%4 (4 heads,
256 output dims). W_q/W_k/W_v are split column-wise per head group, W_o
row-wise; each core computes a partial [S, D] output product which the host
sums per batch (plus the (bv @ Wo.T + bo) row, exact because softmax rows
sum to 1).

v2 pipeline (single fused emission; Tile schedules across engines):
  - Projections of chunk c+1 and the epilogue of chunk c-1 are emitted as
    "filler" groups inside chunk c's attention j-loop, so the PE works
    through them while the ACT engine grinds the exp() stream (the
    attention phase is ACT-bound).
  - Scores matmuls row-pack 2 heads (K=64 each at row groups 0/64).
  - PV matmuls col-pack 2 heads (M=64 each at col groups 0/64) and the
    softmax denominators come from 4 concurrent M=32 ones-matmuls at col
    groups 0/32/64/96 (l_h replicated over 32 partitions), so the whole
    PV+sums step is 3 PE rounds instead of 4.
  - Softmax normalization: 1/l via DVE reciprocal_approx_fast (no Ln/Exp
    table-set ping-pong on ACT; exp is the only ACT function used), then
    a PE ones-outer-product broadcast and one DVE multiply.
  - Output partials are written fp16 (halves the store traffic; host
    accumulates in f32).
"""

import os
import sys

import numpy as np

# concourse (Bass/Tile) normally comes from PYTHONPATH; fall back to the
# container's copy when run from a bare directory.
for _p in ("/root/.axon_site/_ro/trn_rl_repo", "/opt/trn_rl_repo"):
    if _p not in sys.path and os.path.isdir(_p):
        sys.path.append(_p)

S = 2048
D = 1024
HL = 4          # heads per core
DL = 256        # local head dims per core
SC = 512        # sq chunk width
NSC = S // SC   # 4 chunks
KC = D // 128   # 8 contraction chunks for the projections

MM_DTYPE = os.environ.get("BASS_MM_DTYPE", "f16")
TRACE = os.environ.get("BASS_KERNEL_TRACE", "0") == "1"

_CACHE = {}


def _build():
    import concourse.bass as bass
    import concourse.mybir as mybir
    import concourse.tile as tile
    from concourse import bacc

    dt = mybir.dt
    f32 = dt.float32
    mmdt = {"f16": dt.float16, "f32r": dt.float32r, "f32": dt.float32}[MM_DTYPE]

    nc = bacc.Bacc("TRN2", target_bir_lowering=False, debug=False)

    xqT = nc.dram_tensor("xqT", [D, S], mmdt, kind="ExternalInput").ap()
    xkT = nc.dram_tensor("xkT", [D, S], mmdt, kind="ExternalInput").ap()
    xvT = nc.dram_tensor("xvT", [D, S], mmdt, kind="ExternalInput").ap()
    wqT = nc.dram_tensor("wqT", [128, KC, DL], mmdt, kind="ExternalInput").ap()
    wkT = nc.dram_tensor("wkT", [128, KC, DL], mmdt, kind="ExternalInput").ap()
    wvT = nc.dram_tensor("wvT", [128, KC, DL], mmdt, kind="ExternalInput").ap()
    woT = nc.dram_tensor("woT", [128, 2, D], mmdt, kind="ExternalInput").ap()
    bqd = nc.dram_tensor("bqd", [128, 2], f32, kind="ExternalInput").ap()
    bkd = nc.dram_tensor("bkd", [128, 2], f32, kind="ExternalInput").ap()
    maskd = nc.dram_tensor("maskd", [128, 128], mmdt, kind="ExternalInput").ap()
    outd = nc.dram_tensor("out", [S, D], mmdt, kind="ExternalOutput").ap()

    Exp = mybir.ActivationFunctionType.Exp
    Copy = mybir.ActivationFunctionType.Identity

    def mm(ps, lhsT, rhs, start, stop, tile_position=None, skip=False):
        nc.tensor.matmul(
            ps, lhsT, rhs, start=start, stop=stop, tile_position=tile_position,
            skip_group_check=skip,
        )

    xqr = xqT.rearrange("(kc p) s -> p kc s", p=128)
    xkr = xkT.rearrange("(kc p) s -> p kc s", p=128)
    xvr = xvT.rearrange("(kc p) s -> p kc s", p=128)

    with tile.TileContext(nc) as tc:
        with (
            tc.tile_pool(name="const", bufs=1) as constp,
            tc.tile_pool(name="w", bufs=1) as wp,
            tc.tile_pool(name="x", bufs=9) as xp,
            tc.tile_pool(name="pp", bufs=1) as pp,
            tc.tile_pool(name="pt", bufs=4) as ptp,
            tc.tile_pool(name="otr", bufs=4) as orp,
            tc.tile_pool(name="rs", bufs=2) as rsp,
            tc.tile_pool(name="osb", bufs=4) as osp,
            tc.tile_pool(name="psS", bufs=2, space="PSUM") as psS,
            tc.tile_pool(name="psPO", bufs=2, space="PSUM") as psPO,
            tc.tile_pool(name="psSUM", bufs=1, space="PSUM") as psSUM,
            tc.tile_pool(name="psW", bufs=1, space="PSUM") as psW,
        ):
            # ---- constants / weights (DMA order = startup priority) ----
            wq_sb = wp.tile([128, KC, DL], mmdt, tag="wq")
            nc.scalar.dma_start(wq_sb[:], wqT[:])

            def load_x(c):
                ssl = slice(c * SC, (c + 1) * SC)
                xts = []
                for nm, xr in (("q", xqr), ("k", xkr), ("v", xvr)):
                    t = xp.tile([128, KC, SC], mmdt, tag="x", name=f"x{nm}{c}")
                    if c == 0:
                        nc.sync.dma_start(t[:, 0:4, :], xr[:, 0:4, ssl])
                        nc.sync.dma_start(t[:, 4:8, :], xr[:, 4:8, ssl])
                    else:
                        nc.sync.dma_start(t[:], xr[:, :, ssl])
                    xts.append(t)
                return xts

            # chunk-0 x loads interleaved with the remaining weight loads
            mask_sb = constp.tile([128, 1, 128], mmdt, tag="mask")
            nc.sync.dma_start(mask_sb[:, 0, :], maskd[:])
            ssl0 = slice(0, SC)
            xq0 = xp.tile([128, KC, SC], mmdt, tag="x", name="xq0")
            nc.sync.dma_start(xq0[:, 0:4, :], xqr[:, 0:4, ssl0])
            nc.sync.dma_start(xq0[:, 4:8, :], xqr[:, 4:8, ssl0])
            bq_sb = constp.tile([128, 2], f32, tag="bq")
            nc.scalar.dma_start(bq_sb[:], bqd[:])
            wk_sb = wp.tile([128, KC, DL], mmdt, tag="wk")
            nc.scalar.dma_start(wk_sb[:], wkT[:])
            xk0 = xp.tile([128, KC, SC], mmdt, tag="x", name="xk0")
            nc.scalar.dma_start(xk0[:, 0:4, :], xkr[:, 0:4, ssl0])
            nc.scalar.dma_start(xk0[:, 4:8, :], xkr[:, 4:8, ssl0])
            bk_sb = constp.tile([128, 2], f32, tag="bk")
            nc.scalar.dma_start(bk_sb[:], bkd[:])
            wv_sb = wp.tile([128, KC, DL], mmdt, tag="wv")
            nc.scalar.dma_start(wv_sb[:], wvT[:])
            xv0 = xp.tile([128, KC, SC], mmdt, tag="x", name="xv0")
            nc.sync.dma_start(xv0[:, 0:4, :], xvr[:, 0:4, ssl0])
            nc.sync.dma_start(xv0[:, 4:8, :], xvr[:, 4:8, ssl0])
            wo_sb = wp.tile([128, 2, D], mmdt, tag="wo")
            nc.scalar.dma_start(wo_sb[:], woT[:])

            xts_all = {0: [xq0, xk0, xv0]}
            for cc in range(1, NSC):
                xts_all[cc] = load_x(cc)

            ones_f32 = constp.tile([128, 64], f32, tag="ones_f32")
            nc.vector.memset(ones_f32[:], 1.0)
            ones_sb = constp.tile([128, 64], mmdt, tag="ones")
            nc.vector.tensor_copy(ones_sb[:], ones_f32[:])
            wz = constp.tile([128, 512], mmdt, tag="wz")
            nc.vector.memset(wz[:], 0.0)
            wps = psW.tile([128, 512], f32, tag="psw", name="warm")
            for _ in range(12):
                mm(wps[:], wz[:, 0:128], wz[:], start=True, stop=True)

            # ---- persistent activations ----
            QT = [pp.tile([128, S], mmdt, tag=f"qt{t}", name=f"qt{t}") for t in range(2)]
            KT = [pp.tile([128, S], mmdt, tag=f"kt{t}", name=f"kt{t}") for t in range(2)]
            Vt = pp.tile([128, 16, 64 * HL], mmdt, tag="vt")
            OTn = [pp.tile([128, S], mmdt, tag=f"otn{t}", name=f"otn{t}") for t in range(2)]

            # ---- emission helpers ----
            def proj_qk_half(xt, w_sb, b_sb, dstT, t, csl, pool, psz):
                # one output partition-tile of the Q/K projection: 8 (or 16)
                # accumulating matmuls into one psum alloc + bias-add evac
                ps = pool.tile([128, psz], f32, tag=pool_tag(pool), name="ps_qk")
                nts = psz // 512
                for tt in range(nts):
                    for kc in range(KC):
                        mm(
                            ps[:, tt * 512 : tt * 512 + 512],
                            w_sb[:, kc, (t + tt) * 128 : (t + tt + 1) * 128],
                            xt[:, kc, :],
                            start=(kc == 0),
                            stop=(kc == KC - 1),
                        )
                for tt in range(nts):
                    nc.scalar.activation(
                        dstT[t + tt][:, csl],
                        ps[:, tt * 512 : tt * 512 + 512],
                        Copy,
                        bias=b_sb[:, t + tt : t + tt + 1],
                    )

            def pool_tag(pool):
                return {id(psS): "pss", id(psPO): "po", id(psSUM): "sums", id(psW): "psw"}[id(pool)]

            def proj_v_pair(xt, c, pairidx, pool):
                # two 128-seq tiles of the V projection into one psum bank
                ps = pool.tile([128, 512], f32, tag=pool_tag(pool))
                for sub in range(2):
                    sl = slice(sub * 256, sub * 256 + 256)
                    st_loc = pairidx * 2 + sub
                    for kc in range(KC):
                        mm(
                            ps[:, sl],
                            xt[:, kc, st_loc * 128 : (st_loc + 1) * 128],
                            wv_sb[:, kc, :],
                            start=(kc == 0),
                            stop=(kc == KC - 1),
                        )
                for sub in range(2):
                    st = 4 * c + pairidx * 2 + sub
                    nc.scalar.copy(Vt[:, st, :], ps[:, sub * 256 : sub * 256 + 256])

            def proj_closures(c, xts):
                xq, xk, xv = xts
                csl = slice(c * SC, (c + 1) * SC)
                return [
                    lambda t=t: proj_qk_half(xq, wq_sb, bq_sb, QT, t, csl, psW, 512)
                    for t in range(2)
                ] + [
                    lambda t=t: proj_qk_half(xk, wk_sb, bk_sb, KT, t, csl, psW, 512)
                    for t in range(2)
                ] + [
                    lambda p=p: proj_v_pair(xv, c, p, psW)
                    for p in range(2)
                ]

            def emit_scores(c, j):
                d = j - 4 * c
                x0 = max(0, 128 * d)
                pts = []
                for t in range(2):
                    ps = psS.tile([128, 1024], f32, tag="pss")
                    for h2 in range(2):
                        p0 = h2 * 64
                        mm(
                            ps[:, h2 * 512 + x0 : (h2 + 1) * 512],
                            KT[t][p0 : p0 + 64, j * 128 : (j + 1) * 128],
                            QT[t][p0 : p0 + 64, c * SC + x0 : (c + 1) * SC],
                            start=True,
                            stop=True,
                        )
                    pt = ptp.tile([128, 1024], mmdt, tag="pt")
                    psv = ps.rearrange("p (h x) -> p h x", x=512)
                    ptv = pt.rearrange("p (h x) -> p h x", x=512)
                    nc.scalar.activation(ptv[:, :, x0:], psv[:, :, x0:], Exp)
                    if d >= 0:
                        nc.vector.tensor_mul(
                            ptv[:, :, x0 : x0 + 128],
                            ptv[:, :, x0 : x0 + 128],
                            mask_sb[:, 0:1, :].broadcast_to([128, 2, 128]),
                        )
                    pts.append(pt)
                return pts

            def emit_pv(c, j, jmax, pts, po, sums):
                d = j - 4 * c
                x0 = max(0, 128 * d)
                for t in range(2):
                    for h2 in range(2):
                        h = 2 * t + h2
                        mm(
                            po[t][h2 * 64 : h2 * 64 + 64, x0:],
                            Vt[:, j, h * 64 : (h + 1) * 64],
                            pts[t][:, h2 * 512 + x0 : (h2 + 1) * 512],
                            start=(j == 0),
                            stop=(j == jmax),
                            skip=True,
                        )
                for h in range(HL):
                    t, h2 = divmod(h, 2)
                    mm(
                        sums[32 * h : 32 * h + 32, x0:],
                        ones_sb[:, 0:32],
                        pts[t][:, h2 * 512 + x0 : (h2 + 1) * 512],
                        start=(j == 0),
                        stop=(j == jmax),
                        tile_position=(0, 32 * h),
                        skip=True,
                    )

            def chunk_end(c, po, sums, last=False):
                def recip_chain():
                    rf = rsp.tile([128, 512], f32, tag="rf", name=f"rf{c}")
                    nc.vector.reciprocal_approx_fast(rf[:], sums[:])
                    rr = rsp.tile([128, 512], mmdt, tag="rr", name=f"rr{c}")
                    nc.vector.tensor_copy(rr[:], rf[:])
                    return rr

                def po_evac():
                    otrs = []
                    for t in range(2):
                        otr = orp.tile([128, 512], f32, tag="otr", name=f"otr{c}_{t}")
                        nc.vector.tensor_copy(otr[:], po[t][:])
                        otrs.append(otr)
                    return otrs

                if last:
                    rr = recip_chain()
                    otrs = po_evac()
                else:
                    otrs = po_evac()
                    rr = recip_chain()
                return otrs, rr

            def ep_bc(c, t, otrs, rr):
                bc = psW.tile([128, 512], f32, tag="psw")
                for h2 in range(2):
                    h = 2 * t + h2
                    mm(
                        bc[h2 * 64 : h2 * 64 + 64, :],
                        ones_sb[32 * h : 32 * h + 1, :],
                        rr[32 * h : 32 * h + 1, :],
                        start=True,
                        stop=True,
                        tile_position=(32 * h, h2 * 64),
                        skip=True,
                    )
                nc.vector.tensor_mul(
                    OTn[t][:, c * SC : (c + 1) * SC], otrs[t][:], bc[:]
                )

            def ep_outproj(st, n, pool):
                pso = pool.tile([128, 512], f32, tag=pool_tag(pool))
                for k2 in range(2):
                    mm(
                        pso[:],
                        OTn[k2][:, st * 128 : (st + 1) * 128],
                        wo_sb[:, k2, n * 512 : (n + 1) * 512],
                        start=(k2 == 0),
                        stop=(k2 == 1),
                    )
                osb = osp.tile([128, 512], mmdt, tag="osb")
                nc.vector.tensor_copy(osb[:], pso[:])
                nc.sync.dma_start(
                    outd[st * 128 : (st + 1) * 128, n * 512 : (n + 1) * 512], osb[:]
                )

            def ep_closures(c, otrs, rr, pool=psW):
                fns = [lambda t=t: ep_bc(c, t, otrs, rr) for t in range(2)]
                for st in range(4 * c, 4 * c + 4):
                    for n in range(2):
                        fns.append(lambda st=st, n=n, pool=pool: ep_outproj(st, n, pool))
                return fns

            # ---- chunk 0 projections (pipelined across 4 psum pools) ----
            csl0 = slice(0, SC)
            proj_qk_half(xq0, wq_sb, bq_sb, QT, 0, csl0, psS, 1024)
            proj_qk_half(xk0, wk_sb, bk_sb, KT, 0, csl0, psS, 1024)
            proj_v_pair(xv0, 0, 0, psPO)
            proj_v_pair(xv0, 0, 1, psW)

            # ---- main fused loop ----
            ep_state = None  # (otrs, rr) of previous chunk
            for c in range(NSC):
                jmax = 4 * c + 3
                fillers = []
                if ep_state is not None:
                    fillers += ep_closures(c - 1, *ep_state)
                if c + 1 < NSC:
                    fillers += proj_closures(c + 1, xts_all[c + 1])

                po = [
                    psPO.tile([128, 512], f32, tag="po", name=f"po{c}_{t}")
                    for t in range(2)
                ]
                sums = psSUM.tile([128, 512], f32, tag="sums", name=f"sums{c}")

                nj = jmax + 1
                per_j = (len(fillers) + nj - 1) // nj if fillers else 0
                pts_next = emit_scores(c, 0)
                for j in range(nj):
                    pts = pts_next
                    if j < jmax:
                        pts_next = emit_scores(c, j + 1)
                    for _ in range(per_j):
                        if fillers:
                            fillers.pop(0)()
                    emit_pv(c, j, jmax, pts, po, sums)
                ep_state = chunk_end(c, po, sums, last=(c == NSC - 1))

            # ---- tail: last chunk's epilogue (wide groups, psS slots) ----
            otrs3, rr3 = ep_state
            for t in range(2):
                ep_bc(NSC - 1, t, otrs3, rr3)
            for st in range(4 * (NSC - 1), 4 * NSC):
                pso = psS.tile([128, 1024], f32, tag="pss", name=f"pso{st}")
                for n in range(2):
                    for k2 in range(2):
                        mm(
                            pso[:, n * 512 : (n + 1) * 512],
                            OTn[k2][:, st * 128 : (st + 1) * 128],
                            wo_sb[:, k2, n * 512 : (n + 1) * 512],
                            start=(k2 == 0),
                            stop=(k2 == 1),
                        )
                osb = osp.tile([128, D], mmdt, tag="osb2", name=f"osb{st}")
                for n in range(2):
                    half = slice(n * 512, (n + 1) * 512)
                    if (st + n) % 2 == 0:
                        nc.vector.tensor_copy(osb[:, half], pso[:, half])
                    else:
                        nc.scalar.copy(osb[:, half], pso[:, half])
                    nc.sync.dma_start(
                        outd[st * 128 : (st + 1) * 128, half], osb[:, half]
                    )

    nc.compile()
    return nc


def _get_nc():
    key = ("nc", MM_DTYPE)
    if key not in _CACHE:
        _CACHE[key] = _build()
    return _CACHE[key]


def _warr(wT, n):
    """[K, n] -> [128, K//128, n] so the device DMA is contiguous."""
    K = wT.shape[0]
    return np.ascontiguousarray(wT.reshape(K // 128, 128, n).transpose(1, 0, 2))


def make_in_maps(q, k, v, Wq, bq, Wk, bk, Wv, bv, Wo, bo):
    """Host-side shard prep: per-core input dict."""
    f32 = np.float32
    md = {"f16": np.float16, "f32r": f32, "f32": f32}[MM_DTYPE]
    mask = (np.arange(128)[None, :] >= np.arange(128)[:, None]).astype(md)
    # per-batch transposes shared by the 4 cores of each batch
    xqT = [np.ascontiguousarray(q[b].T.astype(md)) for b in range(2)]
    xkT = [np.ascontiguousarray(k[b].T.astype(md)) for b in range(2)]
    xvT = [np.ascontiguousarray(v[b].T.astype(md)) for b in range(2)]
    in_maps = []
    for c in range(8):
        b, g = c // 4, c % 4
        sl = slice(DL * g, DL * (g + 1))
        in_maps.append(
            {
                "xqT": xqT[b],
                "xkT": xkT[b],
                "xvT": xvT[b],
                "wqT": _warr((Wq[sl, :].T * f32(0.125)).astype(md), DL),
                "wkT": _warr(Wk[sl, :].T.astype(md), DL),
                "wvT": _warr(Wv[sl, :].T.astype(md), DL),
                "woT": _warr(Wo[:, sl].T.astype(md), D),
                "bqd": np.ascontiguousarray((bq[sl] * f32(0.125)).reshape(2, 128).T),
                "bkd": np.ascontiguousarray(bk[sl].reshape(2, 128).T),
                "maskd": mask,
            }
        )
    return in_maps


def kernel(q, k, v, Wq, bq, Wk, bk, Wv, bv, Wo, bo):
    from concourse.bass_utils import run_bass_kernel_spmd

    args = [np.asarray(a, dtype=np.float32) for a in (q, k, v, Wq, bq, Wk, bk, Wv, bv, Wo, bo)]
    q, k, v, Wq, bq, Wk, bk, Wv, bv, Wo, bo = args
    nc = _get_nc()
    in_maps = make_in_maps(q, k, v, Wq, bq, Wk, bk, Wv, bv, Wo, bo)
    tmpdir = os.environ.get("BASS_KERNEL_TMPDIR") or None
    res = run_bass_kernel_spmd(nc, in_maps, list(range(8)), trace=TRACE, tmpdir=tmpdir)
    if TRACE and res.exec_time_ns is not None:
        print(f"HW exec time: {res.exec_time_ns} ns")
        print(f"HW exec time mean: {res.mean_exec_time_ns} ns")
    out = np.zeros((2, S, D), np.float32)
    for c in range(8):
        out[c // 4] += res.results[c]["out"].astype(np.float32)
    out += (bv @ Wo.T + bo)[None, None, :]
    return out
